# revision 8
# baseline (speedup 1.0000x reference)
"""Trainium2 Bass kernel for CustomYOLOLoss (N=512, S=52, NB=3), 8-core data parallel.

v2: channel-planar fp16 layout, staged host-side. Each core gets
[128 partitions x 20 planes x 1352 cells] fp16 ("data"), planes ordered so
every engine reads contiguous spans:
  0-5   xy logits (x0,y0,x1,y1,x2,y2)     -> ACT sigmoid [6X]
  6-7   tx,ty
  8-13  wh logits (w0,h0,w1,h1,w2,h2)     -> ACT sigmoid [6X]
  14-15 tw,th
  16-18 conf logits c0,c1,c2              -> ACT softplus(-c) [3X]
  19    tc (exactly 0.0/1.0 -> used directly as the obj mask)

Geometry uses the exact half-width identity (per axis):
  iw_pre = min(s - |d|, min(pw, tw)),  ew = (pw + tw) - iw_pre
with s = pw/2 + tw/2, d = px - tx; inter = relu(iwx)*relu(iwy).
All elementwise work runs in fp16 on the DVE (2x tensor_tensor, 4x
tensor_scalar), transcendentals + |d| + reciprocals on ACT (3 table sets,
loaded once each), masked sums via scalar_tensor_tensor accum_out.

Host-side final math (f64): with per-cell cs = sum_b bce0_b,
bce0 = softplus(c), bce1 = softplus(-c), dd = iou_sel + union_sel/enc_sel:
  S = sum cs, T1 = sum obj*cs, T2NO = sum obj*bce0_sel, NO = sum obj*bce1_sel,
  G = sum obj*dd_sel, NOBJ = sum obj
  loss_noobj = (S-T1)/(3*n_noobj) + (T1-T2NO)/(2*n_obj)
  loss_bbox  = (2*NOBJ - G)/n_obj ; loss_obj = NO/n_obj
"""

import os
import numpy as np

import concourse.bass as bass
import concourse.bacc as bacc
import concourse.mybir as mybir
import concourse.tile as tile
from concourse.bass_utils import run_bass_kernel_spmd

F16 = mybir.dt.float16
F32 = mybir.dt.float32
AF = mybir.ActivationFunctionType
ALU = mybir.AluOpType

N, S, NB = 512, 52, 3
CORES = 8
NPC = N // CORES                      # 64 images per core
P = 128
CELLS = NPC * S * S                   # 173056
X = CELLS // P                        # 1352 cells per partition

# plane offsets (units of X columns) within the [P, 20X] data tensor
PL_XY, PL_TXY, PL_WH, PL_TWH, PL_CC, PL_TC = 0, 6, 8, 14, 16, 19

_nc_cache = {}


def _act_recip(nc, out_ap, in_ap):
    """ACT-engine spline reciprocal. bass.activation() refuses Reciprocal for
    precision reasons irrelevant at this problem's tolerance; emit the
    instruction directly (bias/scale/alpha must be float immediates)."""
    eng = nc.scalar
    imm = lambda v: mybir.ImmediateValue(dtype=mybir.dt.float32, value=v)
    return eng.add_instruction(
        mybir.InstActivation(
            name=eng.bass.get_next_instruction_name(),
            func=AF.Reciprocal,
            ins=[eng.lower_ap(in_ap), imm(0.0), imm(1.0), imm(0.0)],
            outs=[eng.lower_ap(out_ap)],
        )
    )


def build_nc():
    if "nc" in _nc_cache:
        return _nc_cache["nc"]
    nc = bacc.Bacc(trn_type="TRN2", target_bir_lowering=False)
    data = nc.dram_tensor("data", [P, 20 * X], F16, kind="ExternalInput")
    out = nc.dram_tensor("out", [P, 16], F32, kind="ExternalOutput")

    with tile.TileContext(nc) as tc:
        with tc.tile_pool(name="main", bufs=1) as pool:
            g1 = pool.tile([P, 8 * X], F16, tag="g1")   # xy(6) + txy(2)
            g2 = pool.tile([P, 8 * X], F16, tag="g2")   # wh(6) + twth(2)
            g3 = pool.tile([P, 4 * X], F16, tag="g3")   # cc(3) + tc(1)
            sig = pool.tile([P, 12 * X], F16, tag="sig")  # sxy(6) | swh(6)
            bce1 = pool.tile([P, 3 * X], F16, tag="bce1")
            d6 = pool.tile([P, 6 * X], F16, tag="d6")   # d -> |d| -> t1; [0:3X] rue
            s6 = pool.tile([P, 6 * X], F16, tag="s6")   # s -> s2 -> ew
            mn6 = pool.tile([P, 6 * X], F16, tag="mn6")  # mn -> iw (y relu'd)
            inter = pool.tile([P, 3 * X], F16, tag="inter")  # -> iou
            enc = pool.tile([P, 3 * X], F16, tag="enc")      # -> renc -> q -> dd
            aa = pool.tile([P, 3 * X], F16, tag="aa")        # -> union
            areab = pool.tile([P, X], F16, tag="areab")
            m01 = pool.tile([P, X], mybir.dt.uint16, tag="m01")
            m2 = pool.tile([P, X], mybir.dt.uint16, tag="m2")
            acc = pool.tile([P, 16], F32, tag="acc")

            # DMA the three plane groups (independent queues, compute order)
            nc.sync.dma_start(g1[:], data[:, 0:8 * X])
            nc.sync.dma_start(g2[:], data[:, 8 * X:16 * X])
            nc.sync.dma_start(g3[:], data[:, 16 * X:20 * X])

            XYv = lambda t, p0, n: t[:, p0 * X:(p0 + n) * X]
            sxy = sig[:, 0:6 * X]
            swh = sig[:, 6 * X:12 * X]
            txy = XYv(g1, 6, 2)
            th2 = XYv(g2, 6, 2)                     # tw/2, th/2 (in-place scaled)
            cc = XYv(g3, 0, 3)
            tc_pl = XYv(g3, 3, 1)

            # --- ACT phase 1 (sigmoid set): sigmoid + |d| later
            nc.scalar.activation(sxy, g1[:, 0:6 * X], AF.Sigmoid)
            nc.scalar.activation(swh, g2[:, 0:6 * X], AF.Sigmoid)

            # --- DVE: target half-widths in place; d = sxy - txy
            nc.vector.tensor_scalar(th2, th2, 0.5, None, ALU.mult)
            for b in range(NB):
                sl = slice(2 * b * X, (2 * b + 2) * X)
                nc.vector.tensor_tensor(d6[:, sl], sxy[:, sl], txy, ALU.subtract)
            # |d| on ACT (abs lives in every table set)
            nc.scalar.activation(d6[:], d6[:], AF.Abs)

            # s = wh/2 + th2 ; mn = min(2*th2, swh) (per box)
            for b in range(NB):
                sl = slice(2 * b * X, (2 * b + 2) * X)
                nc.vector.scalar_tensor_tensor(
                    s6[:, sl], swh[:, sl], 0.5, th2, ALU.mult, ALU.add)
                nc.vector.scalar_tensor_tensor(
                    mn6[:, sl], th2, 2.0, swh[:, sl], ALU.mult, ALU.min)
            # t1 = s - |d| (into d6) ; iw = min(t1, mn) (into mn6)
            nc.vector.tensor_tensor(d6[:], s6[:], d6[:], ALU.subtract)
            nc.vector.tensor_tensor(mn6[:], d6[:], mn6[:], ALU.min)
            # s2 = 2s (in place) ; ew = s2 - iw (into s6)
            nc.vector.tensor_scalar(s6[:], s6[:], 2.0, None, ALU.mult)
            nc.vector.tensor_tensor(s6[:], s6[:], mn6[:], ALU.subtract)

            # paired (x,y) strided views: [(2X,3),(1,X)]
            pair = lambda ap, o: ap.rearrange("p (b a x) -> p b a x", b=3, a=2)[
                :, :, o, :]
            # relu on iw_y, then inter = relu(iw_x) * iw_y_relu (fused max0)
            nc.vector.tensor_scalar(pair(mn6[:], 1), pair(mn6[:], 1), 0.0, None,
                                    ALU.max)
            nc.vector.scalar_tensor_tensor(
                inter[:], pair(mn6[:], 0), 0.0, pair(mn6[:], 1), ALU.max, ALU.mult)
            # enc = ew_x * ew_y ; aa = sw * sh
            nc.vector.tensor_tensor(enc[:], pair(s6[:], 0), pair(s6[:], 1),
                                    ALU.mult)
            nc.vector.tensor_tensor(aa[:], pair(swh, 0), pair(swh, 1), ALU.mult)
            # areab = (2*th2x)*(2*th2y) = (th2x*4)*th2y
            nc.vector.scalar_tensor_tensor(
                areab[:], g2[:, 6 * X:7 * X], 4.0, g2[:, 7 * X:8 * X],
                ALU.mult, ALU.mult)
            # union = aa + areab - inter (no eps needed: areab >= 2.5e-3)
            for b in range(NB):
                sl = slice(b * X, (b + 1) * X)
                nc.vector.tensor_tensor(aa[:, sl], aa[:, sl], areab[:], ALU.add)
            nc.vector.tensor_tensor(aa[:], aa[:], inter[:], ALU.subtract)

            # --- ACT phase 2 (natural_log_exp set): bce1 = ln(1 + exp(-c))
            # (this build's act tables have no softplus entry)
            nc.scalar.activation(bce1[:], cc, AF.Exp, scale=-1.0)
            nc.scalar.activation(bce1[:], bce1[:], AF.Ln, bias=1.0)

            # --- ACT phase 3 (reciprocal set)
            rue = d6[:, 0:3 * X]                     # d6 dead; reuse
            _act_recip(nc, rue, aa[:])               # 1/union
            _act_recip(nc, enc[:], enc[:])           # 1/enc in place

            # iou = inter * rue ; q = union * renc ; dd = iou + q
            nc.vector.tensor_tensor(inter[:], inter[:], rue, ALU.mult)
            nc.vector.tensor_tensor(enc[:], aa[:], enc[:], ALU.mult)
            nc.vector.tensor_tensor(enc[:], inter[:], enc[:], ALU.add)

            # --- responsible-box masks (strict greater = argmax first-wins)
            mx = d6[:, 4 * X:5 * X]
            iou_b = lambda b: inter[:, b * X:(b + 1) * X]
            nc.vector.tensor_tensor(m01[:], iou_b(1), iou_b(0), ALU.is_gt)
            nc.vector.tensor_tensor(mx, iou_b(0), iou_b(1), ALU.max)
            nc.vector.tensor_tensor(m2[:], iou_b(2), mx, ALU.is_gt)

            # --- bce0 = c + bce1 ; cs = sum_b bce0_b   (g1 dead; reuse)
            bce0 = g1[:, 0:3 * X]
            cs1 = g1[:, 3 * X:4 * X]
            cs = g1[:, 4 * X:5 * X]
            nc.vector.tensor_tensor(bce0, cc, bce1[:], ALU.add)
            nc.vector.tensor_tensor(cs1, g1[:, 0:X], g1[:, X:2 * X], ALU.add)
            nc.vector.tensor_tensor(cs, cs1, g1[:, 2 * X:3 * X], ALU.add)

            # --- select dd / bce0 / bce1 of the responsible box
            sel = g1[:, 5 * X:8 * X]
            for q, t in enumerate((enc[:], bce0, bce1[:])):
                dst = sel[:, q * X:(q + 1) * X]
                nc.vector.tensor_copy(dst, t[:, 0:X])
                nc.vector.copy_predicated(dst, m01[:], t[:, X:2 * X])
                nc.vector.copy_predicated(dst, m2[:], t[:, 2 * X:3 * X])

            # --- masked sums (STT accum: out junk -> g2, acc col)
            junk = lambda k: g2[:, k * X:(k + 1) * X]
            col = lambda k: acc[:, k:k + 1]
            nc.vector.scalar_tensor_tensor(
                junk(0), cs, 1.0, tc_pl, ALU.bypass, ALU.mult, accum_out=col(0))
            nc.vector.scalar_tensor_tensor(
                junk(1), sel[:, 0:X], 1.0, tc_pl, ALU.bypass, ALU.mult,
                accum_out=col(3))
            nc.vector.scalar_tensor_tensor(
                junk(2), sel[:, X:2 * X], 1.0, tc_pl, ALU.bypass, ALU.mult,
                accum_out=col(4))
            nc.vector.scalar_tensor_tensor(
                junk(3), sel[:, 2 * X:3 * X], 1.0, tc_pl, ALU.bypass, ALU.mult,
                accum_out=col(5))
            # S = sum cs, NOBJ = sum tc (ACT copy-accum; copy is in every set)
            nc.scalar.activation(junk(4), cs, AF.Copy, accum_out=col(1))
            nc.scalar.activation(junk(5), tc_pl, AF.Copy, accum_out=col(2))

            nc.gpsimd.dma_start(out[:], acc[:])

    nc.compile()
    _nc_cache["nc"] = nc
    return nc


def _stage(input, target):
    """Full f32 inputs -> per-core planar fp16 [P, 20X] arrays."""
    # [N,S,S,15] -> per-core [P, X, ch] -> [P, ch, X]
    ch = np.empty((N * S * S, 20), dtype=np.float16)
    inp = input.reshape(-1, 15)
    tgt = target.reshape(-1, 5)
    # xy logits
    for b in range(NB):
        ch[:, 2 * b] = inp[:, 5 * b + 1]
        ch[:, 2 * b + 1] = inp[:, 5 * b + 2]
        ch[:, PL_WH + 2 * b] = inp[:, 5 * b + 3]
        ch[:, PL_WH + 2 * b + 1] = inp[:, 5 * b + 4]
        ch[:, PL_CC + b] = inp[:, 5 * b]
    ch[:, PL_TXY] = tgt[:, 1]
    ch[:, PL_TXY + 1] = tgt[:, 2]
    ch[:, PL_TWH] = tgt[:, 3]
    ch[:, PL_TWH + 1] = tgt[:, 4]
    ch[:, PL_TC] = tgt[:, 0]
    percore = ch.reshape(CORES, P, X, 20)
    maps = []
    for c in range(CORES):
        planar = np.ascontiguousarray(percore[c].transpose(0, 2, 1))  # [P,20,X]
        maps.append({"data": planar.reshape(P, 20 * X)})
    return maps


def kernel(input, target):
    nc = build_nc()
    in_maps = _stage(np.asarray(input), np.asarray(target))
    res = run_bass_kernel_spmd(nc, in_maps, core_ids=list(range(CORES)))
    tot = np.zeros(16, dtype=np.float64)
    for r in res.results:
        tot += r["out"].sum(axis=0, dtype=np.float64)
    T1, S_, NOBJ, G, T2NO, NO = tot[0], tot[1], tot[2], tot[3], tot[4], tot[5]
    n_obj = NOBJ
    n_noobj = float(N * S * S) - n_obj
    loss_noobj = (S_ - T1) / (n_noobj * NB) + (T1 - T2NO) / (n_obj * (NB - 1))
    loss_bbox = (2.0 * n_obj - G) / n_obj
    loss_obj = NO / n_obj
    loss = loss_obj + loss_bbox + loss_noobj
    return (np.float32(loss), np.float32(loss_noobj), np.float32(loss_bbox),
            np.float32(loss_obj))
